# revision 2
# baseline (speedup 1.0000x reference)
"""DeepSeekMoE Trainium2 kernel v3 (8 NeuronCores, data-parallel over tokens).

Reference computation (B=128, FEW=64, D=512, E=16, O=512, H=64, K=3):
  t = x.reshape(T=8192, D)
  gates = softmax(relu(t@gW1+gb1)@gW2+gb2)            # [T, E]
  h  = relu(einsum('td,edh->teh', t, W1) + b1)        # [T, E, H]
  eo = einsum('teh,eho->teo', h, W2) + b2             # [T, E, O]
  topv, topi = top_k(gates, 3); out_t = sum_k topv * eo[topi]
  out = mean over FEW  -> [B, 1, 1, O]

v3 key restructure: the FEW-mean commutes into MM2's moving operand
(W2 is token-independent), so
  out[row, o] = sum_h2 W2[h2, o] * hgm[h2, row] + sum_e b2[e, o] * gm[e, row]
with hgm = sum_{t in row} gate*h / 64 and gm = sum_{t in row} gate / 64.
MM2 therefore runs once per core with a 16-column moving operand
(36 matmuls) instead of streaming all 8192 tokens through the PE again
(-15us of PE time vs dense MM2).

Pipeline per 512-token tile:
  PE : gating hidden (fp16, single pass) -> logits -> top-3 mask
       transposes; MM1 pair matmuls interleaved to stay busy; per-pair
       "mask matmul" broadcasts gate rows to the 128 h2 partitions (PSUM)
  DVE: softmax chain; hg = hr * psG (the only PSUM-reading elementwise
       op); small gate-sum reduces
  Act: relu copies (PSUM->SBUF fp16), gmt copies
  Pool: 6-level pairwise-add tree reducing hg [128,512] -> hgm [128,8]
       (SBUF-only, so the otherwise-idle GPSIMD engine can do it)
Final MM2 matmuls stream during the second tile's gate phase; output is
one [128, 4, 16] fp32 DMA, transposed to [16, 512] on the host.

Gating precision: single-pass fp16 hidden layer gives 5/8192 top-3
selection flips and 5.8e-3 end-to-end relative error (tolerance 2e-2).
"""

import sys

import numpy as np

for _p in ("/opt/trn_rl_repo",):
    if _p not in sys.path:
        sys.path.insert(0, _p)

B, FEW, D = 128, 64, 512
E, O, H, TOPK = 16, 512, 64, 3
T = B * FEW            # 8192 tokens
NCORES = 8
TLOC = T // NCORES     # 1024 tokens per core
DT = 512               # tokens per tile
NDT = TLOC // DT       # 2 tiles per core
HALF = DT // 2
PAIRS = E // 2         # 8 expert pairs
OCH = O // 128         # 4 output chunks
NROW = TLOC // FEW     # 16 output rows per core

_CACHE = {}


def _build_nc():
    import concourse.mybir as mybir
    import concourse.tile as tile
    from concourse import bacc

    f32 = mybir.dt.float32
    f16 = mybir.dt.float16
    AF = mybir.ActivationFunctionType
    ALU = mybir.AluOpType
    AX = mybir.AxisListType

    nc = bacc.Bacc("TRN2", target_bir_lowering=False, debug=False,
                   num_devices=NCORES)

    # ---- DRAM I/O ----------------------------------------------------------
    xt16_d = nc.dram_tensor("xt16", [4, 128, TLOC], f16, kind="ExternalInput")
    w1_d = nc.dram_tensor("w1", [128, 4, PAIRS, 128], f16, kind="ExternalInput")
    w2_d = nc.dram_tensor("w2", [128, PAIRS, O], f16, kind="ExternalInput")
    gw1_d = nc.dram_tensor("gw1", [128, 4, H], f16, kind="ExternalInput")
    # small fp32 consts packed: col 0 gb1, cols 1:17 gw2a, cols 17:25 b1
    blob32_d = nc.dram_tensor("blob32", [128, 25], f32, kind="ExternalInput")
    # small fp16 consts packed: cols 0:1024 maskp[16,8,128], 1024:1536 b2t
    cons16_d = nc.dram_tensor("cons16", [E, 1536], f16, kind="ExternalInput")
    id16_d = nc.dram_tensor("id16", [128, 128], f16, kind="ExternalInput")
    out_d = nc.dram_tensor("out", [128, OCH, NROW], f32,
                           kind="ExternalOutput")

    with tile.TileContext(nc) as tc:
        with (
            tc.tile_pool(name="consts", bufs=1) as consts,
            tc.tile_pool(name="work", bufs=3) as work,
            tc.tile_pool(name="psH", bufs=3, space="PSUM") as psH,
            tc.tile_pool(name="psG", bufs=2, space="PSUM") as psG_pool,
            tc.tile_pool(name="psSmall", bufs=2, space="PSUM") as psSmall,
            tc.tile_pool(name="psOut", bufs=1, space="PSUM") as psOut,
            tc.tile_pool(name="scratch", bufs=1) as scratch,
        ):
            # ---- resident SBUF tiles + input DMAs -------------------------
            xt16sb = consts.tile([128, 4, TLOC], f16)
            gw1sb = consts.tile([128, 4, H], f16)
            w1sb = consts.tile([128, 4, PAIRS, 128], f16)
            w2sb = consts.tile([128, PAIRS, O], f16)
            blob32 = consts.tile([128, 25], f32)
            cons16 = consts.tile([E, 1536], f16)
            id16sb = consts.tile([128, 128], f16)
            gb1sb = blob32[0:H, 0:1]
            gw2asb = blob32[0:H + 1, 1:17]
            b1sb = blob32[:, 17:25]
            maskpsb = cons16.rearrange("e (q c) -> e q c", c=128)[:, 0:8, :]
            b2tsb = cons16.rearrange("e (q c) -> e q c", c=128)[:, 8:12, :]

            xt16_r = xt16_d.ap().rearrange("j p t -> p j t")
            nc.sync.dma_start(out=xt16sb[:, :, 0:HALF],
                              in_=xt16_r[:, :, 0:HALF])
            nc.sync.dma_start(out=gw1sb, in_=gw1_d.ap())
            nc.sync.dma_start(out=blob32, in_=blob32_d.ap())
            nc.sync.dma_start(out=xt16sb[:, :, HALF:DT],
                              in_=xt16_r[:, :, HALF:DT])
            nc.sync.dma_start(out=w1sb[:, :, 0:2, :], in_=w1_d.ap()[:, :, 0:2, :])
            nc.sync.dma_start(out=cons16, in_=cons16_d.ap())
            nc.sync.dma_start(out=id16sb, in_=id16_d.ap())
            nc.sync.dma_start(out=w1sb[:, :, 2:8, :], in_=w1_d.ap()[:, :, 2:8, :])
            nc.sync.dma_start(out=w2sb, in_=w2_d.ap())
            nc.sync.dma_start(out=xt16sb[:, :, DT:DT + HALF],
                              in_=xt16_r[:, :, DT:DT + HALF])
            nc.sync.dma_start(out=xt16sb[:, :, DT + HALF:TLOC],
                              in_=xt16_r[:, :, DT + HALF:TLOC])

            # gating hidden activations; row H is the constant-1 row that
            # folds gb2 into the logits matmul
            asb = consts.tile([H + 1, TLOC], f32)
            nc.vector.memset(asb[H:H + 1, :], 1.0)
            # absorb the 1.3us activation-table load while PE waits on DMA
            actwarm = scratch.tile([1, 1], f32)
            nc.scalar.activation(actwarm, asb[H:H + 1, 0:1], AF.Exp, scale=1.0)

            # gate-weighted token means feeding the final MM2, laid out
            # flat as (dti, batch, pair-in-batch, row) so each tree batch
            # writes one contiguous [128, 32] region and each MM2 moving
            # slice [128, 8] is contiguous
            hgmflat = consts.tile([128, 128], f16)
            gmf = consts.tile([E, NROW], f16)
            gm32 = consts.tile([E, NROW], f32)
            outT = consts.tile([128, OCH, NROW], f32)
            psOT = psOut.tile([128, OCH, NROW], f32, name="psOT")

            def mm1_pair(pair, t0, dti):
                """MM1 + relu for one expert pair on a full 512-token tile."""
                psh = psH.tile([128, DT], f32, tag="psh",
                               name=f"psh_{dti}_{pair}")
                for j in range(4):
                    nc.tensor.matmul(psh, w1sb[:, j, pair, :],
                                     xt16sb[:, j, t0:t0 + DT],
                                     start=(j == 0), stop=(j == 3))
                hr = work.tile([128, DT], f16, tag="hr", bufs=6,
                               name=f"hr_{dti}_{pair}")
                nc.scalar.activation(hr, psh, AF.Relu,
                                     bias=b1sb[:, pair:pair + 1], scale=1.0)
                return hr

            def logits_slice(s, t0):
                """Logits matmul + DVE/Act softmax chain for a 128-slice."""
                st = t0 + s * 128
                psL = psSmall.tile([128, E], f32, tag="small")
                nc.tensor.matmul(psL, asb[:, st:st + 128], gw2asb)
                negmax = work.tile([128, 1], f32, tag="negmax")
                nc.vector.tensor_reduce(negmax, psL, axis=AX.X, op=ALU.max,
                                        negate=True)
                expd = work.tile([128, E], f32, tag="expd")
                sume = work.tile([128, 1], f32, tag="sume")
                nc.scalar.activation(expd, psL, AF.Exp, bias=negmax,
                                     scale=1.0, accum_out=sume)
                rsum = work.tile([128, 1], f32, tag="rsum")
                nc.vector.reciprocal(rsum, sume)
                gfull = work.tile([128, E], f32, tag="gfull")
                nc.vector.tensor_scalar_mul(gfull, expd, rsum)
                top8 = work.tile([128, 8], f32, tag="top8")
                nc.vector.max(top8, gfull)
                gmask = work.tile([128, E], f16, tag="gmask")
                nc.vector.scalar_tensor_tensor(gmask, gfull, top8[:, 2:3],
                                               gfull, op0=ALU.is_ge,
                                               op1=ALU.mult)
                return gmask

            def transpose_slice(gmt, gmask, s):
                psGT = psSmall.tile([E, 128], f16, tag="small")
                nc.tensor.transpose(psGT, gmask, id16sb)
                nc.scalar.copy(gmt[:, s * 128:s * 128 + 128], psGT)

            def tree_batch(hg4, bp, dti):
                """Pool pairwise-add tree for 4 pairs at once:
                [128, (4 pairs x 8 rows x 64 tok) flat] -> hgmflat [128, 32].
                hg4 is flat [128, 2048] so every level is a single-split
                rearrange (pool tiles reject grouped views)."""
                src = hg4.rearrange("p (m k) -> p m k", k=64)
                width = 32
                lvl = 1
                while width >= 1:
                    if width > 1:
                        dst = work.tile([128, 32, width], f16,
                                        tag=f"tree{lvl}", bufs=2,
                                        name=f"tr{lvl}_{dti}_{bp}")
                    else:
                        off = dti * 64 + bp * 32
                        dst = hgmflat.rearrange("p (m k) -> p m k",
                                                k=1)[:, off:off + 32, :]
                    eng = nc.vector if lvl == 1 else nc.gpsimd
                    eng.tensor_add(dst, src[:, :, 0:width],
                                   src[:, :, width:2 * width])
                    src = dst
                    width //= 2
                    lvl += 1

            def emit_head(dti):
                """Gating chain + MM1 for one tile as a list of emission
                chunks (each a callable emitting a few PE ops), so callers
                can interleave them between other PE work."""
                t0 = dti * DT
                gmt = work.tile([E, DT], f16, tag="gmt", name=f"gmt_{dti}")
                hrs = []
                gms = {}
                st = {}

                def psa_half(ha):
                    def go():
                        a0 = t0 + ha * HALF
                        psAt = psSmall.tile([H, HALF], f32, tag="small",
                                            name=f"psA_{dti}_{ha}")
                        for j in range(4):
                            nc.tensor.matmul(psAt, gw1sb[:, j, :],
                                             xt16sb[:, j, a0:a0 + HALF],
                                             start=(j == 0), stop=(j == 3))
                        nc.scalar.activation(asb[0:H, a0:a0 + HALF], psAt,
                                             AF.Relu, bias=gb1sb, scale=1.0)
                    return go

                def logits(s):
                    def go():
                        gms[s] = logits_slice(s, t0)
                    return go

                def transp(s):
                    def go():
                        transpose_slice(gmt, gms[s], s)
                    return go

                def mm1(p):
                    def go():
                        hrs.append(mm1_pair(p, t0, dti))
                    return go

                def gmsum():
                    nc.vector.tensor_reduce(
                        gm32[:, dti * 8:(dti + 1) * 8],
                        gmt.rearrange("e (r k) -> e r k", k=64),
                        axis=AX.X, op=ALU.add)
                    nc.scalar.copy(gmf[:, dti * 8:(dti + 1) * 8],
                                   gm32[:, dti * 8:(dti + 1) * 8])

                chunks = [psa_half(0), logits(0), mm1(0), logits(1),
                          transp(0), mm1(1), psa_half(1), transp(1),
                          logits(2), mm1(2), logits(3), transp(2), mm1(3),
                          transp(3), gmsum]
                tail = [mm1(4), mm1(5), mm1(6), mm1(7)]
                return gmt, hrs, chunks, tail

            def emit_gates(dti, gmt, hrs, fillers):
                """8 mask-matmuls + DVE mults + 2 tree batches; pops filler
                chunks between psG matmuls to keep the in-order PE fed while
                the DVE mult chain drains."""
                for bp in range(2):
                    hg4 = work.tile([128, 4 * DT], f16, tag="hg4", bufs=2,
                                    name=f"hg4_{dti}_{bp}")
                    for pi in range(4):
                        pair = 4 * bp + pi
                        psG = psG_pool.tile([128, DT], f32, tag="psG",
                                            name=f"psG_{dti}_{pair}")
                        nc.tensor.matmul(psG, maskpsb[:, pair, :], gmt)
                        nc.vector.tensor_mul(hg4[:, pi * DT:(pi + 1) * DT],
                                             hrs[pair], psG)
                        if fillers:
                            fillers.pop(0)()
                    tree_batch(hg4, bp, dti)
                while fillers:
                    fillers.pop(0)()

            def mm2_chunks(dti, first):
                """MM2 over one tile's 8 output rows as 9 filler chunks
                (4 matmuls each); 8-col moving operands make these nearly
                free on the PE."""
                rs = slice(dti * 8, dti * 8 + 8)

                def b2chunk():
                    for c in range(OCH):
                        nc.tensor.matmul(psOT[:, c, rs], b2tsb[:, c, :],
                                         gmf[:, rs],
                                         start=(first and c == 0),
                                         stop=False)

                def pairchunk(pair):
                    def go():
                        off = dti * 64 + (pair // 4) * 32 + (pair % 4) * 8
                        for c in range(OCH):
                            nc.tensor.matmul(
                                psOT[:, c, rs],
                                w2sb[:, pair, c * 128:(c + 1) * 128],
                                hgmflat[:, off:off + 8],
                                start=False, stop=(pair == PAIRS - 1))
                    return go

                return [b2chunk] + [pairchunk(p) for p in range(PAIRS)]

            # ---- main schedule -------------------------------------------
            gmt0, hrs0, chunks0, tail0 = emit_head(0)
            for ch in chunks0:
                ch()
            gmt1, hrs1, chunks1, tail1 = emit_head(1)
            # DT0 gate phase: fill with DT0's MM1 pairs 4-7 then DT1's head
            emit_gates(0, gmt0, hrs0, tail0 + chunks1)
            # DT1 gate phase: fill with DT1's MM1 pairs 4-7 then DT0's MM2
            emit_gates(1, gmt1, hrs1, tail1 + mm2_chunks(0, True))
            for ch in mm2_chunks(1, False):
                ch()
            nc.vector.tensor_copy(outT.rearrange("p c r -> p (c r)"),
                                  psOT.rearrange("p c r -> p (c r)"))
            nc.sync.dma_start(out=out_d.ap(), in_=outT)

    nc.compile()
    return nc


def _host_inputs(x, gW1, gb1, gW2, gb2, W1, b1, W2, b2):
    """Per-core in_maps with all host-side layout transforms."""
    f = np.float32
    h = np.float16
    xt_full = np.ascontiguousarray(x.reshape(T, D).T.astype(f))       # [D, T]
    # W1 [E,D,H] -> [p, j, pair, s*64+h], e = 2*pair+s, d = 128*j+p
    w1sb = np.ascontiguousarray(
        W1.reshape(PAIRS, 2, 4, 128, H).transpose(3, 2, 0, 1, 4)
        .reshape(128, 4, PAIRS, 128).astype(f))
    # W2 [E,H,O] -> [s*64+h, pair, o], mean folded
    w2sb = np.ascontiguousarray(
        W2.reshape(PAIRS, 2, H, O).transpose(1, 2, 0, 3)
        .reshape(128, PAIRS, O).astype(f) / np.float32(FEW))
    b1sb = np.ascontiguousarray(
        b1.reshape(PAIRS, 2, H).transpose(1, 2, 0).reshape(128, PAIRS).astype(f))
    b2t = (b2.astype(f) / np.float32(FEW)).reshape(E, OCH, 128)
    gw1a = np.ascontiguousarray(
        gW1.reshape(4, 128, H).transpose(1, 0, 2).astype(f))     # [128,4,64]
    gw2a = np.vstack([gW2.astype(f), gb2.reshape(1, E).astype(f)])
    ident = np.eye(128, dtype=f)
    # maskp[e, pair, m] = 1 if e == 2*pair + m//64
    m = np.arange(128)
    pr = np.arange(PAIRS)
    ee = np.arange(E)
    maskp = (ee[:, None, None] == (2 * pr[None, :, None] + m[None, None, :] // 64)
             ).astype(f)

    blob32 = np.zeros((128, 25), dtype=f)
    blob32[0:H, 0] = gb1.astype(f)
    blob32[0:H + 1, 1:17] = gw2a
    blob32[:, 17:25] = b1sb
    cons16 = np.zeros((E, 1536), dtype=h)
    cons16[:, 0:1024] = maskp.reshape(E, 1024).astype(h)
    cons16[:, 1024:1536] = b2t.reshape(E, 512).astype(h)

    shared = dict(w1=w1sb.astype(h), w2=w2sb.astype(h), gw1=gw1a.astype(h),
                  blob32=blob32, cons16=cons16, id16=ident.astype(h))
    in_maps = []
    for c in range(NCORES):
        im = dict(shared)
        xt_c = xt_full[:, c * TLOC:(c + 1) * TLOC]
        im["xt16"] = np.ascontiguousarray(
            xt_c.reshape(4, 128, TLOC).astype(h))
        in_maps.append(im)
    return in_maps


def kernel(x, gW1, gb1, gW2, gb2, W1, b1, W2, b2, _trace=False):
    from concourse.bass_utils import run_bass_kernel_spmd

    if "nc" not in _CACHE:
        _CACHE["nc"] = _build_nc()
    nc = _CACHE["nc"]
    args = [np.asarray(a, dtype=np.float32)
            for a in (x, gW1, gb1, gW2, gb2, W1, b1, W2, b2)]
    in_maps = _host_inputs(*args)
    try:
        kres = run_bass_kernel_spmd(nc, in_maps, core_ids=list(range(NCORES)),
                                    trace=_trace)
    except ModuleNotFoundError:
        kres = run_bass_kernel_spmd(nc, in_maps, core_ids=list(range(NCORES)),
                                    trace=False)
    _CACHE["last_result"] = kres
    # device emits [o_part(128), o_chunk(4), b_row(16)] per core; finish
    # the [o, row] -> [row, o] transpose here
    outs = [np.ascontiguousarray(
                np.transpose(kres.results[c]["out"], (2, 1, 0)).reshape(NROW, O))
            for c in range(NCORES)]
    out = np.concatenate(outs, axis=0)
    return out.reshape(B, 1, 1, O).astype(np.float32)
